# revision 21
# baseline (speedup 1.0000x reference)
"""Multi-head attention (B=8, N=1024, C=768, H=12) on 8 Trainium2 NeuronCores.

Strategy: pure data parallelism over the batch dimension — each of the 8
cores computes full attention for one batch element; weights are
replicated. No collectives needed.

v3 changes vs baseline (221.7us):
  - U^T->token-major and attn->aoT transposes moved off the PE onto the
    DMA XBAR (dma_start_transpose): removes 144 PE transposes + the
    per-token-chunk reciprocal/scalar-mul normalize (96+96 DVE ops) in
    favor of one reciprocal + free-dim-broadcast multiply per head.
  - HAM warmup: dummy matmul stream from t~0 so the PE is at 2.4GHz when
    real matmuls start (baseline ran cold 1.2GHz for its first 38us).
  - x cast-DMA split per 128-token chunk and interleaved with the wq1
    chunk DMAs on the gpsimd queue; x->xT PE transposes start as soon as
    each chunk lands (baseline waited ~17us for one monolithic load).
  - Output DMAs split per 384-col half across two queues, overlap proj.

Per-core dataflow (all matmuls out = lhsT.T @ rhs on the PE):
  1. xs  = cast-DMA(x) bf16 per chunk;  xT = PE-transpose(xs)
  2. qkT = w_qkv[:, :1536].T @ xT    (q,k feature-major)
     v   = x @ w_qkv[:, 1536:]      (v token-major, ones cols 64:80)
  3. per head pair (rows 0:64 = head A, 64:128 = head B of qkT chunks):
       scoresT[m,n] = k_h @ q_h^T
       expT = exp(scale * scoresT)   (ScalarE; max-subtraction skipped:
                                      |scores*scale| < ~2, exp safe)
       U^T[d,n] += v_aug[m,d] expT[m,n]   (rows 64:80 = softmax denom)
       xbar-transpose U^T -> token-major, recip denom, broadcast-mul
  4. aoT = xbar-transpose(attn_out); y = aoT.T @ w_proj + b
"""

import os
import sys

for _p in ("/opt/trn_rl_repo", "/root/.axon_site/_ro/trn_rl_repo"):
    if os.path.isdir(_p) and _p not in sys.path:
        sys.path.append(_p)

from contextlib import ExitStack

import numpy as np

import concourse.bass as bass
import concourse.tile as tile
from concourse import bacc, mybir
from concourse.bass_utils import run_bass_kernel_spmd
from concourse.masks import make_identity

FP = mybir.dt.float32
BF16 = mybir.dt.bfloat16
N_CORES = 8
T = 1024  # tokens per core (batch element)
C = 768
H = 12
D = 64
DU = 80  # U^T rows: 64 dims + denom row 64 + filler ones (16-aligned)
SCALE = D ** (-0.5)
TC = T // 128  # 8 token chunks
CCH = C // 128  # 6 channel chunks
NPAIR = H // 2  # 6 head pairs

Exp = mybir.ActivationFunctionType.Exp
Mult = mybir.AluOpType.mult


def build(n_cores: int = N_CORES):
    MMDT = BF16
    nc = bacc.Bacc(
        "TRN2", target_bir_lowering=False, debug=False, num_devices=n_cores
    )
    x = nc.declare_dram_parameter("x", [T, C], FP, isOutput=False)
    w_qkv = nc.declare_dram_parameter("w_qkv", [C, 3 * C], FP, isOutput=False)
    w_proj = nc.declare_dram_parameter("w_proj", [C, C], FP, isOutput=False)
    b_proj = nc.declare_dram_parameter("b_proj", [C], FP, isOutput=False)
    out = nc.declare_dram_parameter("out", [T, C], FP, isOutput=True)

    xa, wqa, wpa, outa = x.ap(), w_qkv.ap(), w_proj.ap(), out.ap()
    ba = b_proj.ap()
    b_bcast_src = bass.AP(tensor=ba.tensor, offset=ba.offset, ap=[[0, 128]] + ba.ap)

    with tile.TileContext(nc) as tc, ExitStack() as ctx:
        consts = ctx.enter_context(tc.tile_pool(name="consts", bufs=1))
        qk_pool = ctx.enter_context(tc.tile_pool(name="qk", bufs=12))
        v_pool = ctx.enter_context(tc.tile_pool(name="v65", bufs=TC))
        ao_pool = ctx.enter_context(tc.tile_pool(name="attn_out", bufs=1))
        wp_pool = ctx.enter_context(tc.tile_pool(name="wp", bufs=1))
        y_pool = ctx.enter_context(tc.tile_pool(name="y", bufs=4))
        r_pool = ctx.enter_context(tc.tile_pool(name="r", bufs=2))
        xs_pool = ctx.enter_context(tc.tile_pool(name="xstage", bufs=1))
        xT_pool = ctx.enter_context(tc.tile_pool(name="xT", bufs=CCH))
        wq1_pool = ctx.enter_context(tc.tile_pool(name="wq1", bufs=1))
        wq2_pool = ctx.enter_context(tc.tile_pool(name="wq2", bufs=1))
        exp_pool = ctx.enter_context(tc.tile_pool(name="expT", bufs=4))
        uT_pool = ctx.enter_context(tc.tile_pool(name="uT", bufs=2))
        aot_pool = ctx.enter_context(tc.tile_pool(name="aot", bufs=2))
        aoT_pool = ctx.enter_context(tc.tile_pool(name="aoT", bufs=CCH))
        # PSUM: sc 2x2 banks + accA 2x1 + accB 2x1 = 8 banks
        sc_psum = ctx.enter_context(tc.tile_pool(name="sc", bufs=2, space="PSUM"))
        accA = ctx.enter_context(tc.tile_pool(name="accA", bufs=2, space="PSUM"))
        accB = ctx.enter_context(tc.tile_pool(name="accB", bufs=2, space="PSUM"))

        # ---- HAM warmup: keep the PE busy from t~0 through the DMA window
        # so real matmuls run at 2.4GHz. Uses an sc-pool psum slot (scores
        # don't need it until ~13us).
        wt = consts.tile([128, 512], MMDT)
        nc.vector.memset(wt[:], 0.0)
        wps = sc_psum.tile([128, 512], FP, tag="sc", name="sc")
        NWARM = 12
        for i in range(NWARM):
            nc.tensor.matmul(
                wps[:], wt[:, 0:128], wt[:], start=(i == 0), stop=(i == NWARM - 1)
            )
        # dependency-free early ScalarE op: places Activation's block-barrier
        # gather at the head of its queue (else it lands on the first exp,
        # whose deps cycle through DMAs waiting on that same barrier)
        tiny = consts.tile([128, 1], FP)
        nc.scalar.activation(tiny[:], wt[:, 0:1], Exp)

        identity_h = consts.tile([128, 128], MMDT)
        make_identity(nc, identity_h)

        # ---- input DMAs: all casting DMAs ride gpsimd (the only engine
        # that can cast); x chunks first, wq1 j=0/6 interleaved early.
        def grouped(src_ap, width, ngrp, col0):
            row_step = src_ap.ap[0][0]
            return bass.AP(
                tensor=src_ap.tensor,
                offset=src_ap.offset + col0,
                ap=[[row_step, 128], [128 * row_step, ngrp], [1, width]],
            )

        xs_all = xs_pool.tile([128, TC, C], MMDT, tag="xs", name="xs")

        def dma_x(t):
            nc.gpsimd.dma_start(
                xs_all[:, t, :],
                bass.AP(
                    tensor=xa.tensor,
                    offset=xa.offset + t * 128 * C,
                    ap=[[C, 128], [1, C]],
                ),
            )

        wq1_all = wq1_pool.tile([128, CCH, 2 * C], MMDT, tag="wq1", name="wq1")
        wq2_all = wq2_pool.tile([128, CCH, C], MMDT, tag="wq2", name="wq2")
        wp_all = wp_pool.tile([128, CCH, C], MMDT, tag="wp", name="wp")

        def dma_wq1(j):
            lo = j * 128
            nc.gpsimd.dma_start(
                wq1_all[:, :, lo : lo + 128], grouped(wqa, 128, CCH, lo)
            )

        # gpsimd queue order: x0..x3 / wq1(0),wq1(6) woven, then the rest
        dma_x(0)
        dma_x(1)
        dma_wq1(0)
        dma_x(2)
        dma_wq1(6)
        dma_x(3)
        dma_x(4)
        nc.gpsimd.dma_start(wq2_all[:], grouped(wqa, C, CCH, 2 * C))
        dma_x(5)
        dma_x(6)
        dma_x(7)
        f1_order = (0, 6, 1, 7, 2, 8, 3, 9, 4, 10, 5, 11)
        for j in f1_order[2:]:
            dma_wq1(j)
        nc.gpsimd.dma_start(wp_all[:], grouped(wpa, C, CCH, 0))
        b_bcast = consts.tile([128, C], FP)
        nc.sync.dma_start(b_bcast[:], b_bcast_src)
        wq1 = [wq1_all[:, c, :] for c in range(CCH)]
        wq2 = [wq2_all[:, c, :] for c in range(CCH)]
        wp = [wp_all[:, c, :] for c in range(CCH)]

        xT = [xT_pool.tile([128, T], MMDT, tag="xT", name="xT") for _ in range(CCH)]
        v65 = [
            v_pool.tile([128, H, DU], MMDT, tag="v65", name="v65")
            for _ in range(TC)
        ]
        attn2 = ao_pool.tile([128, NPAIR, TC, 128], MMDT, tag="ao", name="ao")
        qkT = [qk_pool.tile([128, T], MMDT, tag="qk", name="qk") for _ in range(12)]
        aoT = [
            aoT_pool.tile([128, TC, 128], MMDT, tag="aoT", name="aoT")
            for _ in range(CCH)
        ]

        def emit_xT(t):
            # x transpose on the PE (idle in this phase anyway)
            xs = xs_all[:, t, :]
            for c in range(CCH):
                ps = accA.tile([128, 512], FP, tag="accA", name="accA")
                psh = ps[:, 0:256].bitcast(MMDT)
                nc.tensor.transpose(
                    psh[:, 0:128], xs[:, c * 128 : (c + 1) * 128], identity_h[:]
                )
                nc.vector.tensor_copy(xT[c][:, t * 128 : (t + 1) * 128], psh[:, 0:128])

        def emit_f1(j, nh):
            # qkT[j][:, nh*512:] = w_qkv[:, j-chunk].T @ x^T half
            ps = accA.tile([128, 512], FP, tag="accA", name="accA")
            for c in range(CCH):
                nc.tensor.matmul(
                    ps[:],
                    wq1[c][:, j * 128 : (j + 1) * 128],
                    xT[c][:, nh * 512 : (nh + 1) * 512],
                    start=(c == 0),
                    stop=(c == CCH - 1),
                )
            nc.vector.tensor_copy(qkT[j][:, nh * 512 : (nh + 1) * 512], ps[:])

        def emit_f2(t):
            # v[t] = x[t-chunk] @ w_qkv[:, v-cols]; ones cols at 64:80
            nc.vector.memset(v65[t][:, :, D:], 1.0)
            for nh in range(2):
                ps = accA.tile([128, 512], FP, tag="accA", name="accA")
                for c in range(CCH):
                    nc.tensor.matmul(
                        ps[:, 0:384],
                        xT[c][:, t * 128 : (t + 1) * 128],
                        wq2[c][:, nh * 384 : (nh + 1) * 384],
                        start=(c == 0),
                        stop=(c == CCH - 1),
                    )
                nc.vector.tensor_copy(
                    v65[t][:, nh * 6 : (nh + 1) * 6, 0:D],
                    ps[:, 0:384].rearrange("p (g d) -> p g d", g=6),
                )

        def emit_scores_exp(p, eAB):
            # half-major: tile_position stays constant for 16 consecutive
            # MMs (config switches cost an exposed ~107ns LDW each)
            for half in range(2):
                base = 64 * half
                for j in range(TC):  # key-token chunks (m)
                    ps = sc_psum.tile([128, T], FP, tag="sc", name="sc")
                    for nh in range(2):
                        nc.tensor.matmul(
                            ps[:, nh * 512 : (nh + 1) * 512],
                            qkT[6 + p][base : base + 64, j * 128 : (j + 1) * 128],
                            qkT[p][base : base + 64, nh * 512 : (nh + 1) * 512],
                            start=True,
                            stop=True,
                        )
                    nc.scalar.activation(
                        eAB[half][:, j, :], ps[:], Exp, scale=SCALE
                    )

        def emit_u(p, eAB):
            # U^T[d, n] = sum_m v_aug[m, d] expT[m, n]; rows 64:80 = denom.
            for half in range(2):
                h = 2 * p + half
                e = eAB[half]
                ups = [
                    accB.tile([DU, 512], FP, tag="accB", name="accB")
                    for _ in range(2)
                ]
                for j in range(TC):  # lhsT (v) reused across both nh
                    for nh in range(2):
                        nc.tensor.matmul(
                            ups[nh][:],
                            v65[j][:, h, :],
                            e[:, j, nh * 512 : (nh + 1) * 512],
                            start=(j == 0),
                            stop=(j == TC - 1),
                        )
                # evict (fp32->bf16), xbar-transpose to token-major,
                # normalize by 1/denom with a free-dim broadcast multiply
                uT_sb = uT_pool.tile([DU, T], MMDT, tag="uT", name="uT")
                nc.vector.tensor_copy(uT_sb[:, 0:512], ups[0][:])
                nc.vector.tensor_copy(uT_sb[:, 512:1024], ups[1][:])
                ao_t = aot_pool.tile([128, TC, DU], MMDT, tag="aot", name="aot")
                nc.sync.dma_start_transpose(ao_t[:], uT_sb[:])
                r = r_pool.tile([128, TC, 1], FP, tag="r", name="r")
                nc.vector.reciprocal(r[:], ao_t[:, :, D : D + 1])
                eng = nc.vector if half == 0 else nc.gpsimd
                eng.tensor_tensor(
                    attn2[:, p, :, half * D : (half + 1) * D],
                    ao_t[:, :, 0:D],
                    r[:].to_broadcast([128, TC, D]),
                    Mult,
                )
            # attn pair block -> feature-major proj lhsT (one xbar transpose)
            nc.sync.dma_start_transpose(aoT[p][:], attn2[:, p, :, :])

        def emit_proj():
            # proj: y = aoT.T @ w_proj + b ; output DMA per 384-col half.
            # Staged: the first 6 groups run their c=0..4 accumulation MMs
            # before any c=5 MM, so the PE streams ready work while the
            # last pair's normalize chain (xbar+recip+mul+xbar) completes.
            groups = [(t, nh) for t in range(TC) for nh in range(2)]
            pss = {}

            def grp_pool(g):
                k3 = g % 6
                pool = (accA, accA, accB, accB, sc_psum, sc_psum)[k3]
                tg = ("accA", "accA", "accB", "accB", "sc", "sc")[k3]
                return pool, tg

            def grp_head(g):
                t, nh = groups[g]
                pool, tg = grp_pool(g)
                ps = pool.tile([128, 512], FP, tag=tg, name=tg)
                pss[g] = ps
                for c in range(CCH - 1):
                    nc.tensor.matmul(
                        ps[:, 0:384],
                        aoT[c][:, t, :],
                        wp[c][:, nh * 384 : (nh + 1) * 384],
                        start=(c == 0),
                        stop=False,
                    )

            def grp_tail(g):
                t, nh = groups[g]
                ps = pss.pop(g)
                c = CCH - 1
                nc.tensor.matmul(
                    ps[:, 0:384],
                    aoT[c][:, t, :],
                    wp[c][:, nh * 384 : (nh + 1) * 384],
                    start=False,
                    stop=True,
                )
                y = y_pool.tile([128, 384], FP, tag="y", name="y")
                nc.vector.tensor_add(
                    y[:], ps[:, 0:384], b_bcast[:, nh * 384 : (nh + 1) * 384]
                )
                eng = nc.sync if nh == 0 else nc.gpsimd
                eng.dma_start(
                    outa[t * 128 : (t + 1) * 128, nh * 384 : (nh + 1) * 384],
                    y[:],
                )

            for g in range(6):
                grp_head(g)
            for g in range(6, len(groups)):
                grp_tail(g - 6)
                grp_head(g)
            for g in range(len(groups) - 6, len(groups)):
                grp_tail(g)

        # ---- woven emission schedule ----
        for t in range(4):
            emit_xT(t)
        emit_f1(0, 0)
        emit_f1(6, 0)
        for t in range(4, TC):
            emit_xT(t)
        emit_f1(0, 1)
        emit_f1(6, 1)
        eAB_list = []

        def new_pair():
            eAB = [
                exp_pool.tile([128, TC, T], MMDT, tag="expT", name="expT")
                for _ in range(2)
            ]
            eAB_list.append(eAB)
            return eAB

        emit_scores_exp(0, new_pair())
        for t in range(TC):
            emit_f2(t)
        emit_f1(1, 0)
        emit_f1(1, 1)
        emit_f1(7, 0)
        emit_f1(7, 1)
        emit_scores_exp(1, new_pair())
        for p in range(2, NPAIR):
            emit_f1(f1_order[2 * p], 0)
            emit_f1(f1_order[2 * p], 1)
            emit_f1(f1_order[2 * p + 1], 0)
            emit_f1(f1_order[2 * p + 1], 1)
            emit_scores_exp(p, new_pair())
            emit_u(p - 2, eAB_list[p - 2])
        emit_u(NPAIR - 2, eAB_list[NPAIR - 2])
        emit_u(NPAIR - 1, eAB_list[NPAIR - 1])
        emit_proj()

    nc.finalize()
    return nc


_NC_CACHE = {}


def _get_nc():
    if "nc" not in _NC_CACHE:
        _NC_CACHE["nc"] = build()
    return _NC_CACHE["nc"]


def kernel(x, w_qkv, w_proj, b_proj):
    """Full inputs in, full output out. Shards batch across 8 NeuronCores."""
    assert x.shape == (N_CORES, T, C), x.shape
    nc = _get_nc()
    in_maps = [
        {
            "x": np.ascontiguousarray(x[i], dtype=np.float32),
            "w_qkv": np.ascontiguousarray(w_qkv, dtype=np.float32),
            "w_proj": np.ascontiguousarray(w_proj, dtype=np.float32),
            "b_proj": np.ascontiguousarray(b_proj, dtype=np.float32),
        }
        for i in range(N_CORES)
    ]
    res = run_bass_kernel_spmd(nc, in_maps, list(range(N_CORES)))
    return np.stack([res.results[i]["out"] for i in range(N_CORES)], axis=0)
